# revision 7
# baseline (speedup 1.0000x reference)
"""CRPS loss kernel for Trainium2, data-parallel over 8 NeuronCores.

Math per (sample, timestep): sorted quantiles q_0..q_10, target y.
With r_j = q_j - y, sgn_j = sign(r_j), the quantile-score weights
m2_j = (j - 10*[r_j >= 0])^2 = (5*sgn_j + (5-j))^2  (exact small fp16 ints;
sign(0)=0 differs from the reference only on measure-zero exact ties).

Summation-by-parts form of the reference trapezoid integral:

  CRPS = (1/200) * [ sum_k q_k * dm_k  -  2y*(m2_0 - m2_10) ]
  dm_k = m2_{k-1} - m2_{k+1}   (clamped: m2_{-1}:=m2_0, m2_11:=m2_10)

so the central-difference runs over fp16 m2 (DVE 2x mode) instead of fp32 q,
and all edge/y terms collapse into one small contiguous op.
out[n] = mean_t CRPS(n, t).

Engine split per block (128 samples x 512 t x 11 j):
  SP  : q DMA + y DMA
  Pool: r = q - y (back share; stride/broadcast-blind engine), 2 strided
        dm edge-column fixes, md = m2_0 - m2_10 (contiguous out)
  DVE : r front share, u = 5*sgn + (5-j) (fp16 stt, 2x), flat fp16 dm
        central-diff (2x), grand reduce sum(q*dm) via tensor_tensor_reduce,
        y-term stt with accum
  ACT : sgn = Sign(r), m2 = Square(u)
"""
import sys

if "/opt/trn_rl_repo" not in sys.path:
    sys.path.insert(0, "/opt/trn_rl_repo")

import numpy as np
import concourse.bass as bass
import concourse.tile as tile
from concourse import bacc, mybir
from concourse.bass_utils import run_bass_kernel_spmd
from concourse.alu_op_type import AluOpType

N_CORES = 8
N, T, D = 4096, 512, 11
N_LOC = N // N_CORES        # 512 samples per core
P = 128                     # partitions
BLOCKS = N_LOC // P         # 4
FP = mybir.dt.float32
F16 = mybir.dt.float16
ACT = mybir.ActivationFunctionType

FD = T * D                  # 5632 flat elements per partition per block
Q16 = False                 # ship q to HBM as fp16 (halves DMA, 2x reduce)
QDT = F16 if Q16 else FP
R_SPLIT_T = 96              # leading t-rows of r computed on DVE (rest Pool)


def build_block(nc, b, inp_r, tgt_r, pools, j5b, s1a):
    qpool, mpool, spool, apool = pools
    tg32 = spool.tile([P, T], FP, tag="tg32")
    nc.sync.dma_start(tg32[:], tgt_r[b]).annotate(f"tdma{b}")
    qt = qpool.tile([P, FD], QDT, tag="qt")
    nc.sync.dma_start(qt[:], inp_r[b]).annotate(f"qdma{b}")

    q3 = qt[:].rearrange("p (t i) -> p t i", i=D)
    yb = tg32[:].unsqueeze(2).broadcast_to([P, T, D])
    j5bb = j5b[:].unsqueeze(1).broadcast_to([P, T, D])

    # r = q - y (fp16), split DVE front / Pool back
    r = mpool.tile([P, FD], F16, tag="r")
    r3 = r[:].rearrange("p (t i) -> p t i", i=D)
    s = R_SPLIT_T
    nc.vector.tensor_tensor(
        r3[:, :s], q3[:, :s], yb[:, :s], AluOpType.subtract
    ).annotate(f"rv{b}")
    nc.gpsimd.tensor_tensor(
        r3[:, s:], q3[:, s:], yb[:, s:], AluOpType.subtract
    ).annotate(f"rp{b}")

    # u = 5*sign(r) + (5-j)  -> m2 = u^2 (both in fp16; ACT then DVE then ACT)
    nc.scalar.activation(r[:], r[:], ACT.Sign).annotate(f"sgn{b}")
    nc.vector.scalar_tensor_tensor(
        r3, r3, 5.0, j5bb, AluOpType.mult, AluOpType.add
    ).annotate(f"u{b}")
    m2 = mpool.tile([P, FD], F16, tag="m2")
    m3 = m2[:].rearrange("p (t i) -> p t i", i=D)
    nc.scalar.activation(m2[:], r[:], ACT.Square).annotate(f"sq{b}")

    # dm = clamped central diff of m2: flat fp16 pass + 2 strided edge fixes
    dm = mpool.tile([P, FD], F16, tag="dm")
    dm3 = dm[:].rearrange("p (t i) -> p t i", i=D)
    nc.vector.scalar_tensor_tensor(
        dm[:, 1 : FD - 1], m2[:, 0 : FD - 2], 1.0, m2[:, 2:FD],
        AluOpType.mult, AluOpType.subtract,
    ).annotate(f"dmflat{b}")
    nc.gpsimd.tensor_tensor(
        dm3[:, :, 0:1], m3[:, :, 0:1], m3[:, :, 1:2], AluOpType.subtract
    ).annotate(f"dmc0_{b}")
    nc.gpsimd.tensor_tensor(
        dm3[:, :, 10:11], m3[:, :, 9:10], m3[:, :, 10:11], AluOpType.subtract
    ).annotate(f"dmc10_{b}")

    # md = m2_0 - m2_10 (contiguous [P,T] fp16) for the y-term
    md = spool.tile([P, T], F16, tag="md")
    nc.gpsimd.tensor_tensor(
        md[:], m3[:, :, 0:1].squeeze(2), m3[:, :, 10:11].squeeze(2),
        AluOpType.subtract,
    ).annotate(f"md{b}")

    # S = sum(q * dm) (product written in place over dm; stt carries accum)
    nc.vector.scalar_tensor_tensor(
        dm[:], dm[:], 1.0, qt[:], AluOpType.mult, AluOpType.mult,
        accum_out=s1a[:, 2 * b : 2 * b + 1],
    ).annotate(f"prod{b}")
    # y-term: sum(-2y * md)
    nc.vector.scalar_tensor_tensor(
        md[:], tg32[:], -2.0, md[:], AluOpType.mult, AluOpType.mult,
        accum_out=s1a[:, 2 * b + 1 : 2 * b + 2],
    ).annotate(f"yterm{b}")


def build_crps_kernel(tc, out_ap, inp_ap, tgt_ap, pools, j5b):
    nc = tc.nc
    qpool, mpool, spool, apool = pools
    inp_r = inp_ap.rearrange("(b p) t i -> b p (t i)", p=P)   # [4, 128, 5632]
    tgt_r = tgt_ap.rearrange("(b p) t -> b p t", p=P)          # [4, 128, 512]

    s1a = apool.tile([P, 2 * BLOCKS], FP, tag="s1a")
    for b in range(BLOCKS):
        build_block(nc, b, inp_r, tgt_r, pools, j5b, s1a)

    # out = (S + yterm) / (200 * T) per block
    s2 = s1a[:].rearrange("p (b two) -> p b two", two=2)
    r1 = apool.tile([P, BLOCKS], FP, tag="r1")
    nc.vector.tensor_tensor(
        r1[:].unsqueeze(2), s2[:, :, 0:1], s2[:, :, 1:2], AluOpType.add
    ).annotate("rsum")
    r2 = apool.tile([P, BLOCKS], FP, tag="r2")
    nc.vector.tensor_scalar_mul(r2[:], r1[:], 1.0 / (200.0 * T)).annotate("rscale")
    nc.sync.dma_start(out_ap.rearrange("(b p) -> p b", p=P), r2[:])


def _build_nc(repeat=1):
    nc = bacc.Bacc("TRN2", target_bir_lowering=False, debug=False,
                   num_devices=N_CORES)
    inp = nc.dram_tensor("inp", [N_LOC, T, D], QDT, kind="ExternalInput").ap()
    tgt = nc.dram_tensor("target", [N_LOC, T], FP, kind="ExternalInput").ap()
    out = nc.dram_tensor("out", [N_LOC], FP, kind="ExternalOutput").ap()
    unroll = next(u for u in (8, 4, 2, 1) if repeat % u == 0)
    with tile.TileContext(nc) as tc:
        with (
            tc.tile_pool(name="qp", bufs=3) as qpool,
            tc.tile_pool(name="mp", bufs=3) as mpool,
            tc.tile_pool(name="sp", bufs=3) as spool,
            tc.tile_pool(name="acc", bufs=2) as apool,
            tc.tile_pool(name="cst", bufs=1) as cpool,
        ):
            pools = (qpool, mpool, spool, apool)
            # constant tile [P, 11] = (5 - j) = 5,4,...,-5 (exact in fp16)
            j5b = cpool.tile([P, D], F16, tag="j5b")
            nc.gpsimd.iota(j5b[:], pattern=[[-1, D]], base=5,
                           channel_multiplier=0,
                           allow_small_or_imprecise_dtypes=True)
            if repeat == 1:
                build_crps_kernel(tc, out, inp, tgt, pools, j5b)
            else:
                with tc.For_i(0, repeat // unroll, 1, staggered_reset=True):
                    for _ in range(unroll):
                        build_crps_kernel(tc, out, inp, tgt, pools, j5b)
    nc.compile()
    return nc


_NC_CACHE = {}


def get_nc(repeat=1):
    if repeat not in _NC_CACHE:
        _NC_CACHE[repeat] = _build_nc(repeat)
    return _NC_CACHE[repeat]


def kernel(inp: np.ndarray, target: np.ndarray) -> np.ndarray:
    inp = np.ascontiguousarray(inp, dtype=mybir.dt.np(QDT))
    target = np.ascontiguousarray(target, dtype=np.float32)
    nc = get_nc()
    in_maps = [
        {
            "inp": inp[c * N_LOC : (c + 1) * N_LOC],
            "target": target[c * N_LOC : (c + 1) * N_LOC],
        }
        for c in range(N_CORES)
    ]
    res = run_bass_kernel_spmd(nc, in_maps, core_ids=list(range(N_CORES)))
    return np.concatenate([res.results[c]["out"] for c in range(N_CORES)])


# revision 8
# speedup vs baseline: 1.2500x; 1.2500x over previous
"""CRPS loss kernel for Trainium2, data-parallel over 8 NeuronCores.

Math per (sample, timestep): sorted quantiles q_0..q_10, target y,
o_j = [q_j >= y], m2_j = (j - 10*o_j)^2 (exact small fp16 ints, computed by
one fused custom DVE op). Summation-by-parts form of the reference
trapezoid integral:

  CRPS = (1/200) * [ sum_k q_k * dm_k  -  2y*(m2_0 - m2_10) ]
  dm_k = m2_{k-1} - m2_{k+1}   (clamped: m2_{-1}:=m2_0, m2_11:=m2_10)

The central difference runs over fp16 m2 (plain tensor_tensor, the only
fast DVE shape: ~0.78 ns/elem) instead of fp32 q, and the edge/y terms
collapse into one [P,T]-sized op. out[n] = mean_t CRPS(n, t).

Engine split per block (128 samples x 512 t x 11 j), HW-measured costs:
  SP  : q DMA 2.9MB + y DMA (~12us spread over 16 queues)
  DVE : fused m2 custom op (8.8us), flat fp16 dm diff (4.4us),
        front slice of prod (plain TT, in place on dm)
  Pool: strided dm edge-column fixes, md = m2_0-m2_10, y-term product,
        back slice of prod (stride-blind, ~1.9 ns/elem)
  ACT : grand accumulation of prod (Copy+accum, 5us), y-term accumulation
        with scale=-2 folded in
"""
import sys

if "/opt/trn_rl_repo" not in sys.path:
    sys.path.insert(0, "/opt/trn_rl_repo")

import numpy as np
import concourse.bass as bass
import concourse.tile as tile
from concourse import bacc, mybir
from concourse import dve_ops as _dve_ops
from concourse.dve_spec import Spec, Src0, Src1, C0, C1, Zero, PageIdx, Idx, sq, lower
from concourse.dve_uop import DveOpSpec
from concourse.bass_utils import run_bass_kernel_spmd
from concourse.alu_op_type import AluOpType

N_CORES = 8
N, T, D = 4096, 512, 11
N_LOC = N // N_CORES        # 512 samples per core
P = 128                     # partitions
BLOCKS = N_LOC // P         # 4
FP = mybir.dt.float32
F16 = mybir.dt.float16
ACT = mybir.ActivationFunctionType

FD = T * D                  # 5632 flat elements per partition per block
PROD_T = 88                 # leading t-rows of prod on DVE (rest on Pool)


def _register_crps_op():
    """Fused m2 = sq(in-page-idx - C0*[q >= y]) custom DVE op (idempotent)."""
    name = "CRPS_SEL_SQ2"
    for op in _dve_ops.OPS:
        if op.name == name:
            return op
    jm = Idx - PageIdx(Zero, C1)
    body = sq(jm - C0 * (Src0 >= Src1))

    def _ref(in0, in1, c0, c1, c2):
        x = np.asarray(in0, np.float32)
        j = np.arange(x.shape[2], dtype=np.float32)[None, None, :]
        c0v = c0 if isinstance(c0, float) else np.asarray(c0, np.float32).reshape(-1, 1, 1)
        yb = np.broadcast_to(np.asarray(in1, np.float32), x.shape)
        o = (x >= yb).astype(np.float32)
        return (j - c0v * o) ** 2

    spec = Spec(body=body, reference=_ref)
    row = 1 + len(_dve_ops.OPS)
    _dve_ops._SUB_OPCODE_FOR_NAME[name] = row
    shas = {}
    for ver in ("v3", "v4"):
        s = DveOpSpec(name=name, opcode=row, uops=lower(spec, ver=ver), rd1_en=True)
        shas[ver] = s.sha(ver)
    op = _dve_ops.DveOp(name, spec, subdim=True, uops_sha=shas)
    _dve_ops.OPS.append(op)
    _dve_ops.CUSTOM_DVE_SPECS[name] = spec
    return op


CRPS_SEL_SQ2 = _register_crps_op()


def build_block(nc, b, inp_r, tgt_r, pools, s1a):
    qpool, mpool, spool, apool = pools
    tg32 = spool.tile([P, T], FP, tag="tg32")
    nc.sync.dma_start(tg32[:], tgt_r[b]).annotate(f"tdma{b}")
    qt = qpool.tile([P, FD], FP, tag="qt")
    nc.sync.dma_start(qt[:], inp_r[b]).annotate(f"qdma{b}")

    q3 = qt[:].rearrange("p (t i) -> p t i", i=D)
    yb = tg32[:].unsqueeze(2).broadcast_to([P, T, D])

    # m2 = (j - 10*[q >= y])^2 (fp16, fused custom op on DVE)
    m2 = mpool.tile([P, FD], F16, tag="m2")
    m3 = m2[:].rearrange("p (t i) -> p t i", i=D)
    nc.vector._custom_dve(
        CRPS_SEL_SQ2, out=m3, in0=q3, in1=yb, s0=10.0, s1=float(D),
    ).annotate(f"m2_{b}")

    # dm = clamped central diff of m2: flat fp16 TT + 2 strided edge fixes
    dm = mpool.tile([P, FD], F16, tag="dm")
    dm3 = dm[:].rearrange("p (t i) -> p t i", i=D)
    nc.vector.tensor_tensor(
        dm[:, 1 : FD - 1], m2[:, 0 : FD - 2], m2[:, 2:FD], AluOpType.subtract
    ).annotate(f"dmflat{b}")
    nc.gpsimd.tensor_tensor(
        dm3[:, :, 0:1], m3[:, :, 0:1], m3[:, :, 1:2], AluOpType.subtract
    ).annotate(f"dmc0_{b}")
    nc.gpsimd.tensor_tensor(
        dm3[:, :, 10:11], m3[:, :, 9:10], m3[:, :, 10:11], AluOpType.subtract
    ).annotate(f"dmc10_{b}")

    # md = m2_0 - m2_10 (contiguous [P,T] fp16), then ytp = md * y in place
    md = spool.tile([P, T], F16, tag="md")
    nc.gpsimd.tensor_tensor(
        md[:], m3[:, :, 0:1].squeeze(2), m3[:, :, 10:11].squeeze(2),
        AluOpType.subtract,
    ).annotate(f"md{b}")
    nc.gpsimd.tensor_tensor(md[:], md[:], tg32[:], AluOpType.mult).annotate(f"ytp{b}")

    # prod = q * dm in place on dm: front rows DVE, back rows Pool
    s = PROD_T
    nc.vector.tensor_tensor(
        dm3[:, :s], dm3[:, :s], q3[:, :s], AluOpType.mult
    ).annotate(f"prodv{b}")
    nc.gpsimd.tensor_tensor(
        dm3[:, s:], dm3[:, s:], q3[:, s:], AluOpType.mult
    ).annotate(f"prodp{b}")

    # accumulate on ACT: S = sum(prod) (scratch over m2, now dead),
    # yterm = sum(-2 * ytp) (scratch over tg32, now dead)
    nc.scalar.activation(
        m2[:], dm[:], ACT.Copy, accum_out=s1a[:, 2 * b : 2 * b + 1]
    ).annotate(f"sacc{b}")
    nc.scalar.activation(
        tg32[:], md[:], ACT.Copy, scale=-2.0,
        accum_out=s1a[:, 2 * b + 1 : 2 * b + 2],
    ).annotate(f"yacc{b}")


def build_crps_kernel(tc, out_ap, inp_ap, tgt_ap, pools):
    nc = tc.nc
    qpool, mpool, spool, apool = pools
    inp_r = inp_ap.rearrange("(b p) t i -> b p (t i)", p=P)   # [4, 128, 5632]
    tgt_r = tgt_ap.rearrange("(b p) t -> b p t", p=P)          # [4, 128, 512]

    s1a = apool.tile([P, 2 * BLOCKS], FP, tag="s1a")
    for b in range(BLOCKS):
        build_block(nc, b, inp_r, tgt_r, pools, s1a)

    # out = (S + yterm) / (200 * T) per block
    s2 = s1a[:].rearrange("p (b two) -> p b two", two=2)
    r1 = apool.tile([P, BLOCKS], FP, tag="r1")
    nc.vector.tensor_tensor(
        r1[:].unsqueeze(2), s2[:, :, 0:1], s2[:, :, 1:2], AluOpType.add
    ).annotate("rsum")
    r2 = apool.tile([P, BLOCKS], FP, tag="r2")
    nc.vector.tensor_scalar_mul(r2[:], r1[:], 1.0 / (200.0 * T)).annotate("rscale")
    nc.sync.dma_start(out_ap.rearrange("(b p) -> p b", p=P), r2[:])


def _build_nc(repeat=1):
    nc = bacc.Bacc("TRN2", target_bir_lowering=False, debug=False,
                   num_devices=N_CORES)
    inp = nc.dram_tensor("inp", [N_LOC, T, D], FP, kind="ExternalInput").ap()
    tgt = nc.dram_tensor("target", [N_LOC, T], FP, kind="ExternalInput").ap()
    out = nc.dram_tensor("out", [N_LOC], FP, kind="ExternalOutput").ap()
    unroll = next(u for u in (8, 4, 2, 1) if repeat % u == 0)
    with tile.TileContext(nc) as tc:
        with (
            tc.tile_pool(name="qp", bufs=3) as qpool,
            tc.tile_pool(name="mp", bufs=3) as mpool,
            tc.tile_pool(name="sp", bufs=3) as spool,
            tc.tile_pool(name="acc", bufs=2) as apool,
        ):
            pools = (qpool, mpool, spool, apool)
            if repeat == 1:
                build_crps_kernel(tc, out, inp, tgt, pools)
            else:
                with tc.For_i(0, repeat // unroll, 1, staggered_reset=True):
                    for _ in range(unroll):
                        build_crps_kernel(tc, out, inp, tgt, pools)
    nc.compile()
    return nc


_NC_CACHE = {}


def get_nc(repeat=1):
    if repeat not in _NC_CACHE:
        _NC_CACHE[repeat] = _build_nc(repeat)
    return _NC_CACHE[repeat]


def kernel(inp: np.ndarray, target: np.ndarray) -> np.ndarray:
    inp = np.ascontiguousarray(inp, dtype=np.float32)
    target = np.ascontiguousarray(target, dtype=np.float32)
    nc = get_nc()
    in_maps = [
        {
            "inp": inp[c * N_LOC : (c + 1) * N_LOC],
            "target": target[c * N_LOC : (c + 1) * N_LOC],
        }
        for c in range(N_CORES)
    ]
    res = run_bass_kernel_spmd(nc, in_maps, core_ids=list(range(N_CORES)))
    return np.concatenate([res.results[c]["out"] for c in range(N_CORES)])


# revision 10
# speedup vs baseline: 1.3486x; 1.0789x over previous
"""CRPS loss kernel for Trainium2, data-parallel over 8 NeuronCores.

Math per (sample, timestep): sorted quantiles q_0..q_10, target y,
o_j = [q_j >= y], m2_j = (j - 10*o_j)^2 (exact small fp16 ints, computed by
one fused custom DVE op). Summation-by-parts form of the reference
trapezoid integral:

  CRPS = (1/200) * [ sum_k q_k * dm_k  -  2y*(m2_0 - m2_10) ]
  dm_k = m2_{k-1} - m2_{k+1}   (clamped: m2_{-1}:=m2_0, m2_11:=m2_10)

The central difference runs over fp16 m2 (plain tensor_tensor, the only
fast DVE shape: ~0.78 ns/elem) instead of fp32 q, and the edge/y terms
collapse into one [P,T]-sized op. out[n] = mean_t CRPS(n, t).

Engine split per block (128 samples x 512 t x 11 j), HW-measured costs:
  SP  : q DMA 2.9MB + y DMA (~12us spread over 16 queues)
  DVE : fused m2 custom op (8.8us), flat fp16 dm diff (4.4us),
        front slice of prod (plain TT, in place on dm)
  Pool: strided dm edge-column fixes, md = m2_0-m2_10, y-term product,
        back slice of prod (stride-blind, ~1.9 ns/elem)
  ACT : grand accumulation of prod (Copy+accum, 5us), y-term accumulation
        with scale=-2 folded in
"""
import sys

if "/opt/trn_rl_repo" not in sys.path:
    sys.path.insert(0, "/opt/trn_rl_repo")

import numpy as np
import concourse.bass as bass
import concourse.tile as tile
from concourse import bacc, mybir
from concourse import dve_ops as _dve_ops
from concourse.dve_spec import Spec, Src0, Src1, C0, C1, Zero, PageIdx, Idx, sq, lower
from concourse.dve_uop import DveOpSpec
from concourse.bass_utils import run_bass_kernel_spmd
from concourse.alu_op_type import AluOpType

N_CORES = 8
N, T, D = 4096, 512, 11
N_LOC = N // N_CORES        # 512 samples per core
P = 128                     # partitions
BLOCKS = N_LOC // P         # 4
FP = mybir.dt.float32
F16 = mybir.dt.float16
ACT = mybir.ActivationFunctionType

FD = T * D                  # 5632 flat elements per partition per block
PROD_T = 286                 # leading t-rows of prod on DVE (rest on Pool)


def _register_crps_op():
    """Fused m2 = sq(in-page-idx - C0*[q >= y]) custom DVE op (idempotent)."""
    name = "CRPS_SEL_SQ2"
    for op in _dve_ops.OPS:
        if op.name == name:
            return op
    jm = Idx - PageIdx(Zero, C1)
    body = sq(jm - C0 * (Src0 >= Src1))

    def _ref(in0, in1, c0, c1, c2):
        x = np.asarray(in0, np.float32)
        j = np.arange(x.shape[2], dtype=np.float32)[None, None, :]
        c0v = c0 if isinstance(c0, float) else np.asarray(c0, np.float32).reshape(-1, 1, 1)
        yb = np.broadcast_to(np.asarray(in1, np.float32), x.shape)
        o = (x >= yb).astype(np.float32)
        return (j - c0v * o) ** 2

    spec = Spec(body=body, reference=_ref)
    row = 1 + len(_dve_ops.OPS)
    _dve_ops._SUB_OPCODE_FOR_NAME[name] = row
    shas = {}
    for ver in ("v3", "v4"):
        s = DveOpSpec(name=name, opcode=row, uops=lower(spec, ver=ver), rd1_en=True)
        shas[ver] = s.sha(ver)
    op = _dve_ops.DveOp(name, spec, subdim=True, uops_sha=shas)
    _dve_ops.OPS.append(op)
    _dve_ops.CUSTOM_DVE_SPECS[name] = spec
    return op


CRPS_SEL_SQ2 = _register_crps_op()


def build_block(nc, b, inp_r, tgt_r, pools, s1a):
    qpool, mpool, spool, apool = pools
    tg32 = spool.tile([P, T], FP, tag="tg32")
    nc.sync.dma_start(tg32[:], tgt_r[b]).annotate(f"tdma{b}")
    qt = qpool.tile([P, FD], FP, tag="qt")
    nc.sync.dma_start(qt[:], inp_r[b]).annotate(f"qdma{b}")

    q3 = qt[:].rearrange("p (t i) -> p t i", i=D)
    yb = tg32[:].unsqueeze(2).broadcast_to([P, T, D])

    # m2 = (j - 10*[q >= y])^2 (fp16, fused custom op on DVE)
    m2 = mpool.tile([P, FD], F16, tag="m2")
    m3 = m2[:].rearrange("p (t i) -> p t i", i=D)
    nc.vector._custom_dve(
        CRPS_SEL_SQ2, out=m3, in0=q3, in1=yb, s0=10.0, s1=float(D),
    ).annotate(f"m2_{b}")

    # dm = clamped central diff of m2: flat fp16 TT + 2 strided edge fixes
    dmt = mpool.tile([P, FD + 2], F16, tag="dm")
    dm = dmt[:, 1 : FD + 1]
    dm3 = dm.rearrange("p (t i) -> p t i", i=D)
    nc.vector.tensor_tensor(
        dmt[:, 2:FD], m2[:, 0 : FD - 2], m2[:, 2:FD], AluOpType.subtract
    ).annotate(f"dmflat{b}")
    nc.gpsimd.tensor_tensor(
        dm3[:, :, 0:1], m3[:, :, 0:1], m3[:, :, 1:2], AluOpType.subtract
    ).annotate(f"dmc0_{b}")
    nc.gpsimd.tensor_tensor(
        dm3[:, :, 10:11], m3[:, :, 9:10], m3[:, :, 10:11], AluOpType.subtract
    ).annotate(f"dmc10_{b}")

    # md = m2_0 - m2_10 (contiguous [P,T] fp16), then ytp = md * y in place
    md = spool.tile([P, T], F16, tag="md")
    nc.gpsimd.tensor_tensor(
        md[:], m3[:, :, 0:1].squeeze(2), m3[:, :, 10:11].squeeze(2),
        AluOpType.subtract,
    ).annotate(f"md{b}")
    nc.gpsimd.tensor_tensor(md[:], md[:], tg32[:], AluOpType.mult).annotate(f"ytp{b}")

    # prod = q * dm in place on dm: front rows DVE, back rows Pool
    s = PROD_T
    nc.vector.tensor_tensor(
        dm3[:, :s], dm3[:, :s], q3[:, :s], AluOpType.mult
    ).annotate(f"prodv{b}")
    nc.gpsimd.tensor_tensor(
        dm3[:, s:], dm3[:, s:], q3[:, s:], AluOpType.mult
    ).annotate(f"prodp{b}")

    # accumulate on ACT: S = sum(prod) (scratch over m2, now dead),
    # yterm = sum(-2 * ytp) (scratch over tg32, now dead)
    nc.scalar.activation(
        m2[:], dm, ACT.Copy, accum_out=s1a[:, b : b + 1]
    ).annotate(f"sacc{b}")
    nc.scalar.activation(
        tg32[:], md[:], ACT.Copy, scale=-2.0,
        accum_out=s1a[:, BLOCKS + b : BLOCKS + b + 1],
    ).annotate(f"yacc{b}")


def build_crps_kernel(tc, out_ap, inp_ap, tgt_ap, pools):
    nc = tc.nc
    qpool, mpool, spool, apool = pools
    inp_r = inp_ap.rearrange("(b p) t i -> b p (t i)", p=P)   # [4, 128, 5632]
    tgt_r = tgt_ap.rearrange("(b p) t -> b p t", p=P)          # [4, 128, 512]

    s1a = apool.tile([P, 2 * BLOCKS], FP, tag="s1a")
    for b in range(BLOCKS):
        build_block(nc, b, inp_r, tgt_r, pools, s1a)

    # out = (S + yterm) / (200 * T) per block
    r1 = apool.tile([P, BLOCKS], FP, tag="r1")
    nc.vector.tensor_tensor(
        r1[:], s1a[:, 0:BLOCKS], s1a[:, BLOCKS : 2 * BLOCKS], AluOpType.add
    ).annotate("rsum")
    r2 = apool.tile([P, BLOCKS], FP, tag="r2")
    nc.vector.tensor_scalar_mul(r2[:], r1[:], 1.0 / (200.0 * T)).annotate("rscale")
    nc.sync.dma_start(out_ap.rearrange("(b p) -> p b", p=P), r2[:])


def _build_nc(repeat=1):
    nc = bacc.Bacc("TRN2", target_bir_lowering=False, debug=False,
                   num_devices=N_CORES)
    inp = nc.dram_tensor("inp", [N_LOC, T, D], FP, kind="ExternalInput").ap()
    tgt = nc.dram_tensor("target", [N_LOC, T], FP, kind="ExternalInput").ap()
    out = nc.dram_tensor("out", [N_LOC], FP, kind="ExternalOutput").ap()
    unroll = next(u for u in (8, 4, 2, 1) if repeat % u == 0)
    with tile.TileContext(nc) as tc:
        with (
            tc.tile_pool(name="qp", bufs=3) as qpool,
            tc.tile_pool(name="mp", bufs=3) as mpool,
            tc.tile_pool(name="sp", bufs=3) as spool,
            tc.tile_pool(name="acc", bufs=2) as apool,
        ):
            pools = (qpool, mpool, spool, apool)
            if repeat == 1:
                build_crps_kernel(tc, out, inp, tgt, pools)
            else:
                with tc.For_i(0, repeat // unroll, 1, staggered_reset=True):
                    for _ in range(unroll):
                        build_crps_kernel(tc, out, inp, tgt, pools)
    nc.compile()
    return nc


_NC_CACHE = {}


def get_nc(repeat=1):
    if repeat not in _NC_CACHE:
        _NC_CACHE[repeat] = _build_nc(repeat)
    return _NC_CACHE[repeat]


def kernel(inp: np.ndarray, target: np.ndarray) -> np.ndarray:
    inp = np.ascontiguousarray(inp, dtype=np.float32)
    target = np.ascontiguousarray(target, dtype=np.float32)
    nc = get_nc()
    in_maps = [
        {
            "inp": inp[c * N_LOC : (c + 1) * N_LOC],
            "target": target[c * N_LOC : (c + 1) * N_LOC],
        }
        for c in range(N_CORES)
    ]
    res = run_bass_kernel_spmd(nc, in_maps, core_ids=list(range(N_CORES)))
    return np.concatenate([res.results[c]["out"] for c in range(N_CORES)])


# revision 11
# speedup vs baseline: 1.3520x; 1.0025x over previous
"""CRPS loss kernel for Trainium2, data-parallel over 8 NeuronCores.

Math per (sample, timestep): sorted quantiles q_0..q_10, target y,
o_j = [q_j >= y], m2_j = (j - 10*o_j)^2 (exact small fp16 ints, computed by
one fused custom DVE op). Summation-by-parts form of the reference
trapezoid integral:

  CRPS = (1/200) * [ sum_k q_k * dm_k  -  2y*(m2_0 - m2_10) ]
  dm_k = m2_{k-1} - m2_{k+1}   (clamped: m2_{-1}:=m2_0, m2_11:=m2_10)

The central difference runs over fp16 m2 (plain tensor_tensor, the only
fast DVE shape: ~0.78 ns/elem) instead of fp32 q, and the edge/y terms
collapse into one [P,T]-sized op. out[n] = mean_t CRPS(n, t).

Engine split per block (128 samples x 512 t x 11 j), HW-measured costs:
  SP  : q DMA 2.9MB + y DMA (~12us spread over 16 queues)
  DVE : fused m2 custom op (8.8us), flat fp16 dm diff (4.4us),
        front slice of prod (plain TT, in place on dm)
  Pool: strided dm edge-column fixes, md = m2_0-m2_10, y-term product,
        back slice of prod (stride-blind, ~1.9 ns/elem)
  ACT : grand accumulation of prod (Copy+accum, 5us), y-term accumulation
        with scale=-2 folded in
"""
import sys

if "/opt/trn_rl_repo" not in sys.path:
    sys.path.insert(0, "/opt/trn_rl_repo")

import numpy as np
import concourse.bass as bass
import concourse.tile as tile
from concourse import bacc, mybir
from concourse import dve_ops as _dve_ops
from concourse.dve_spec import Spec, Src0, Src1, C0, C1, Zero, PageIdx, Idx, sq, lower
from concourse.dve_uop import DveOpSpec
from concourse.bass_utils import run_bass_kernel_spmd
from concourse.alu_op_type import AluOpType

N_CORES = 8
N, T, D = 4096, 512, 11
N_LOC = N // N_CORES        # 512 samples per core
P = 128                     # partitions
BLOCKS = N_LOC // P         # 4
FP = mybir.dt.float32
F16 = mybir.dt.float16
ACT = mybir.ActivationFunctionType

FD = T * D                  # 5632 flat elements per partition per block
PROD_T = 286                 # leading t-rows of prod on DVE (rest on Pool)


def _register_crps_op():
    """Fused m2 = sq(in-page-idx - C0*[q >= y]) custom DVE op (idempotent)."""
    name = "CRPS_SEL_SQ2"
    for op in _dve_ops.OPS:
        if op.name == name:
            return op
    jm = Idx - PageIdx(Zero, C1)
    body = sq(jm - C0 * (Src0 >= Src1))

    def _ref(in0, in1, c0, c1, c2):
        x = np.asarray(in0, np.float32)
        j = np.arange(x.shape[2], dtype=np.float32)[None, None, :]
        c0v = c0 if isinstance(c0, float) else np.asarray(c0, np.float32).reshape(-1, 1, 1)
        yb = np.broadcast_to(np.asarray(in1, np.float32), x.shape)
        o = (x >= yb).astype(np.float32)
        return (j - c0v * o) ** 2

    spec = Spec(body=body, reference=_ref)
    row = 1 + len(_dve_ops.OPS)
    _dve_ops._SUB_OPCODE_FOR_NAME[name] = row
    shas = {}
    for ver in ("v3", "v4"):
        s = DveOpSpec(name=name, opcode=row, uops=lower(spec, ver=ver), rd1_en=True)
        shas[ver] = s.sha(ver)
    op = _dve_ops.DveOp(name, spec, subdim=True, uops_sha=shas)
    _dve_ops.OPS.append(op)
    _dve_ops.CUSTOM_DVE_SPECS[name] = spec
    return op


CRPS_SEL_SQ2 = _register_crps_op()


def build_block(nc, b, inp_r, tgt_r, pools, s1a):
    qpool, mpool, spool, apool = pools
    tg32 = spool.tile([P, T], FP, tag="tg32")
    nc.sync.dma_start(tg32[:], tgt_r[b]).annotate(f"tdma{b}")
    qt = qpool.tile([P, FD], FP, tag="qt")
    nc.sync.dma_start(qt[:], inp_r[b]).annotate(f"qdma{b}")

    q3 = qt[:].rearrange("p (t i) -> p t i", i=D)
    yb = tg32[:].unsqueeze(2).broadcast_to([P, T, D])

    # m2 = (j - 10*[q >= y])^2 (fp16, fused custom op on DVE)
    m2 = mpool.tile([P, FD], F16, tag="m2")
    m3 = m2[:].rearrange("p (t i) -> p t i", i=D)
    nc.vector._custom_dve(
        CRPS_SEL_SQ2, out=m3, in0=q3, in1=yb, s0=10.0, s1=float(D),
    ).annotate(f"m2_{b}")

    # dm = clamped central diff of m2: flat fp16 TT + 2 strided edge fixes
    dmt = mpool.tile([P, FD + 2], F16, tag="dm")
    dm = dmt[:, 1 : FD + 1]
    dm3 = dm.rearrange("p (t i) -> p t i", i=D)
    nc.vector.tensor_tensor(
        dmt[:, 2:FD], m2[:, 0 : FD - 2], m2[:, 2:FD], AluOpType.subtract
    ).annotate(f"dmflat{b}")
    nc.gpsimd.tensor_tensor(
        dm3[:, :, 0:1], m3[:, :, 0:1], m3[:, :, 1:2], AluOpType.subtract
    ).annotate(f"dmc0_{b}")
    nc.gpsimd.tensor_tensor(
        dm3[:, :, 10:11], m3[:, :, 9:10], m3[:, :, 10:11], AluOpType.subtract
    ).annotate(f"dmc10_{b}")

    # md = m2_0 - m2_10 (contiguous [P,T] fp16), then ytp = md * y in place
    md = spool.tile([P, T], F16, tag="md")
    nc.gpsimd.tensor_tensor(
        md[:], m3[:, :, 0:1].squeeze(2), m3[:, :, 10:11].squeeze(2),
        AluOpType.subtract,
    ).annotate(f"md{b}")
    nc.gpsimd.tensor_tensor(md[:], md[:], tg32[:], AluOpType.mult).annotate(f"ytp{b}")

    # prod = q * dm in place on dm: front rows DVE, back rows Pool
    s = PROD_T
    nc.vector.tensor_tensor(
        dm3[:, :s], dm3[:, :s], q3[:, :s], AluOpType.mult
    ).annotate(f"prodv{b}")
    nc.gpsimd.tensor_tensor(
        dm3[:, s:], dm3[:, s:], q3[:, s:], AluOpType.mult
    ).annotate(f"prodp{b}")

    # accumulate on ACT: S = sum(prod) (scratch over m2, now dead),
    # yterm = sum(-2 * ytp) (scratch over tg32, now dead)
    nc.scalar.activation(
        m2[:], dm, ACT.Copy, accum_out=s1a[:, b : b + 1]
    ).annotate(f"sacc{b}")
    nc.scalar.activation(
        tg32[:], md[:], ACT.Copy, scale=-2.0,
        accum_out=s1a[:, BLOCKS + b : BLOCKS + b + 1],
    ).annotate(f"yacc{b}")


def build_crps_kernel(tc, out_ap, inp_ap, tgt_ap, pools):
    nc = tc.nc
    qpool, mpool, spool, apool = pools
    inp_r = inp_ap.rearrange("(b p) t i -> b p (t i)", p=P)   # [4, 128, 5632]
    tgt_r = tgt_ap.rearrange("(b p) t -> b p t", p=P)          # [4, 128, 512]

    s1a = apool.tile([P, 2 * BLOCKS], FP, tag="s1a")
    for b in range(BLOCKS):
        build_block(nc, b, inp_r, tgt_r, pools, s1a)

    # out = (S + yterm) / (200 * T) per block
    r1 = apool.tile([P, BLOCKS], FP, tag="r1")
    nc.vector.tensor_tensor(
        r1[:], s1a[:, 0:BLOCKS], s1a[:, BLOCKS : 2 * BLOCKS], AluOpType.add
    ).annotate("rsum")
    r2 = apool.tile([P, BLOCKS], FP, tag="r2")
    nc.vector.tensor_scalar_mul(r2[:], r1[:], 1.0 / (200.0 * T)).annotate("rscale")
    nc.sync.dma_start(out_ap.rearrange("(b p) -> p b", p=P), r2[:])


def _build_nc(repeat=1):
    nc = bacc.Bacc("TRN2", target_bir_lowering=False, debug=False,
                   num_devices=N_CORES)
    inp = nc.dram_tensor("inp", [N_LOC, T, D], FP, kind="ExternalInput").ap()
    tgt = nc.dram_tensor("target", [N_LOC, T], FP, kind="ExternalInput").ap()
    out = nc.dram_tensor("out", [N_LOC], FP, kind="ExternalOutput").ap()
    unroll = next(u for u in (8, 4, 2, 1) if repeat % u == 0)
    with tile.TileContext(nc) as tc:
        with (
            tc.tile_pool(name="qp", bufs=4) as qpool,
            tc.tile_pool(name="mp", bufs=4) as mpool,
            tc.tile_pool(name="sp", bufs=4) as spool,
            tc.tile_pool(name="acc", bufs=2) as apool,
        ):
            pools = (qpool, mpool, spool, apool)
            if repeat == 1:
                build_crps_kernel(tc, out, inp, tgt, pools)
            else:
                with tc.For_i(0, repeat // unroll, 1, staggered_reset=True):
                    for _ in range(unroll):
                        build_crps_kernel(tc, out, inp, tgt, pools)
    nc.compile()
    return nc


_NC_CACHE = {}


def get_nc(repeat=1):
    if repeat not in _NC_CACHE:
        _NC_CACHE[repeat] = _build_nc(repeat)
    return _NC_CACHE[repeat]


def kernel(inp: np.ndarray, target: np.ndarray) -> np.ndarray:
    inp = np.ascontiguousarray(inp, dtype=np.float32)
    target = np.ascontiguousarray(target, dtype=np.float32)
    nc = get_nc()
    in_maps = [
        {
            "inp": inp[c * N_LOC : (c + 1) * N_LOC],
            "target": target[c * N_LOC : (c + 1) * N_LOC],
        }
        for c in range(N_CORES)
    ]
    res = run_bass_kernel_spmd(nc, in_maps, core_ids=list(range(N_CORES)))
    return np.concatenate([res.results[c]["out"] for c in range(N_CORES)])
